# revision 22
# baseline (speedup 1.0000x reference)
"""Trainium2 Bass kernel for nn_Encoder (recursive GRU-merge encoder).

Data-parallel over batch: B=128 examples -> 8 NeuronCores x 16 examples.
Everything runs on-core in a single NEFF: 14 full reduce iterations
(2-step GRU over adjacent pairs, argmax pair selection, attention over the
relation embedding table, entropy loss, merge+compact), the final L=2 GRU,
and the final attention.

Layouts (per core, Bl=16 local examples, E=512, R=1000):
  - activations column-major "folded": [128 part, 4 E-tiles, L(j), 16(b)]
    j-major free layout so valid column ranges stay contiguous.
  - GRU pre-activations live in PSUM: pG [128, 12, 16, 16] (6 banks).
    gh2 (h1 @ W_hh.T) PE-accumulates onto pG for the r/z gates (shifted
    slot trick: h1 of pair p is stored at slot j=p+1, so the accumulate
    target is the contiguous range j=1..L-1).
  - biases are added with ones-row matmuls into PSUM, so gate activations
    are single folded ACT ops reading PSUM directly.
"""

import numpy as np

B, S, E, R = 128, 16, 512, 1000
NCORES = 8
BL = B // NCORES          # 16 examples per core
ET = 4                    # E // 128
E3 = 3 * E                # 1536
RT = 8                    # ceil(R/128)
RPAD = RT * 128           # 1024
SCALE = 1.0 / np.sqrt(E)

_CACHE = {}


def _build(debug_dump=False):
    """Build and compile the Bass module once."""
    import concourse.mybir as mybir
    import concourse.tile as tile
    from concourse import bacc
    from concourse.bass import ds
    from concourse.masks import make_identity

    FP = mybir.dt.float32
    AX = mybir.AxisListType
    OP = mybir.AluOpType
    AF = mybir.ActivationFunctionType

    nc = bacc.Bacc("TRN2", target_bir_lowering=False, debug=False,
                   enable_asserts=False, num_devices=NCORES)

    # ---------------- DRAM I/O ----------------
    d_x0 = nc.dram_tensor("x0T", [ET, 128, S, BL], FP, kind="ExternalInput")
    d_wih = nc.dram_tensor("wihT", [ET, 128, E3], FP, kind="ExternalInput")
    d_whh = nc.dram_tensor("whhT", [ET, 128, E3], FP, kind="ExternalInput")
    d_wqk = nc.dram_tensor("wqkT", [ET, 128, 2 * E], FP, kind="ExternalInput")
    d_embR = nc.dram_tensor("embR", [RT, 128, E], FP, kind="ExternalInput")
    d_embT = nc.dram_tensor("embT", [ET, 128, R], FP, kind="ExternalInput")
    d_browG = nc.dram_tensor("browG", [1, E3], FP, kind="ExternalInput")
    d_brownh = nc.dram_tensor("brownh", [1, E], FP, kind="ExternalInput")
    d_bnh = nc.dram_tensor("bnhcol", [128, ET], FP, kind="ExternalInput")
    d_bqk = nc.dram_tensor("bqk", [128, 2 * ET], FP, kind="ExternalInput")
    d_wfc = nc.dram_tensor("wfc", [128, ET], FP, kind="ExternalInput")

    d_scores = nc.dram_tensor("scores", [BL, R + 1], FP, kind="ExternalOutput")
    d_losses = nc.dram_tensor("losses", [BL, S], FP, kind="ExternalOutput")
    if debug_dump:
        d_dbg = {
            "dbg_h1s": nc.dram_tensor("dbg_h1s", [128, ET, S - 1, BL], FP,
                                      kind="ExternalOutput"),
            "dbg_h2": nc.dram_tensor("dbg_h2", [128, ET, S - 1, BL], FP,
                                     kind="ExternalOutput"),
            "dbg_sel": nc.dram_tensor("dbg_sel", [1, BL], FP,
                                      kind="ExternalOutput"),
            "dbg_pair": nc.dram_tensor("dbg_pair", [128, ET, BL], FP,
                                       kind="ExternalOutput"),
            "dbg_e": nc.dram_tensor("dbg_e", [BL, R], FP,
                                    kind="ExternalOutput"),
            "dbg_d": nc.dram_tensor("dbg_d", [BL, R], FP,
                                    kind="ExternalOutput"),
            "dbg_mm": nc.dram_tensor("dbg_mm", [BL, 1], FP,
                                     kind="ExternalOutput"),
            "dbg_mgd": nc.dram_tensor("dbg_mgd", [128, ET, BL], FP,
                                      kind="ExternalOutput"),
            "dbg_y": nc.dram_tensor("dbg_y", [128, ET, S - 1, BL], FP,
                                    kind="ExternalOutput"),
            "dbg_ppre": nc.dram_tensor("dbg_ppre", [128, 12, S - 1, BL], FP,
                                       kind="ExternalOutput"),
            "dbg_pn": nc.dram_tensor("dbg_pn", [128, ET, S - 1, BL], FP,
                                     kind="ExternalOutput"),
            "dbg_pre0": nc.dram_tensor("dbg_pre0", [128, 8, S - 1, BL], FP,
                                       kind="ExternalOutput"),
        }

    with tile.TileContext(nc) as tc:
        from contextlib import ExitStack
        _pools = ExitStack()
        pers = _pools.enter_context(tc.tile_pool(name="pers", bufs=1))

        def _tct(shape, dtype, name):
            return pers.tile(shape, dtype, tag=name, name=name)
        # ---------------- persistent SBUF ----------------
        wih_sb = _tct([128, ET, E3], FP, name="wih_sb")
        whh_sb = _tct([128, ET, E3], FP, name="whh_sb")
        wqk_sb = _tct([128, ET, 2 * E], FP, name="wqk_sb")
        embR_sb = _tct([128, RT, E], FP, name="embR_sb")
        browG_sb = _tct([1, E3], FP, name="browG_sb")
        brownh_sb = _tct([1, E], FP, name="brownh_sb")
        bnh_sb = _tct([128, ET], FP, name="bnh_sb")
        bqk_sb = _tct([128, 2 * ET], FP, name="bqk_sb")
        wfc_sb = _tct([128, ET], FP, name="wfc_sb")
        krelT_sb = _tct([128, ET, R], FP, name="krelT_sb")
        ident = _tct([128, 128], FP, name="ident")
        ones_r = _tct([1, BL * S], FP, name="ones_r")     # [1,256] of 1.0
        ones_r1 = _tct([1, 128], FP, name="ones_r1")      # [1,128] of 1.0
        ones_c = _tct([128, 1], FP, name="ones_c")        # [128,1] of 1.0
        eT_sb = _tct([128, RT, BL], FP, name="eT_sb")
        Zs = _tct([BL, S], FP, name="Zs")
        Ts = _tct([BL, S], FP, name="Ts")
        losses_sb = _tct([BL, S], FP, name="losses_sb")

        # input DMAs (split so compute can start as soon as possible)
        for t in range(ET):
            nc.sync.dma_start(wih_sb[:, t, :], d_wih[t])
        nc.sync.dma_start(browG_sb[:], d_browG[:])
        nc.sync.dma_start(brownh_sb[:], d_brownh[:])
        nc.sync.dma_start(bnh_sb[:], d_bnh[:])
        nc.sync.dma_start(bqk_sb[:], d_bqk[:])
        nc.sync.dma_start(wfc_sb[:], d_wfc[:])
        for t in range(ET):
            nc.sync.dma_start(whh_sb[:, t, :], d_whh[t])
        for t in range(ET):
            nc.sync.dma_start(wqk_sb[:, t, :], d_wqk[t])
        for t in range(RT):
            nc.sync.dma_start(embR_sb[:, t, :], d_embR[t])

        make_identity(nc, ident[:])
        nc.gpsimd.memset(ones_r[:], 1.0)
        nc.gpsimd.memset(ones_r1[:], 1.0)
        nc.gpsimd.memset(ones_c[:], 1.0)
        nc.gpsimd.memset(eT_sb[96:128, RT - 1, :], 0.0)
        nc.gpsimd.memset(Zs[:], 1.0)
        nc.gpsimd.memset(Ts[:], 0.0)

        # ---------------- pools ----------------
        xp = _pools.enter_context(tc.tile_pool(name="xp", bufs=3))
        gw = _pools.enter_context(tc.tile_pool(name="gw", bufs=1))
        h2p = _pools.enter_context(tc.tile_pool(name="h2p", bufs=2))
        selp = _pools.enter_context(tc.tile_pool(name="selp", bufs=1))
        att = _pools.enter_context(tc.tile_pool(name="att", bufs=2))
        lsp = _pools.enter_context(tc.tile_pool(name="lsp", bufs=1))
        pp = _pools.enter_context(tc.tile_pool(name="pp", bufs=1, space="PSUM"))

        # ------ k_rel.T = (emb[:R] @ Wk.T + bk).T  [E, R], uses embT ------
        with tc.tile_pool(name="boot", bufs=1) as bp:
            embT_sb = bp.tile([128, ET, R], FP, name="embT_sb")
            for t in range(ET):
                nc.sync.dma_start(embT_sb[:, t, :], d_embT[t])
            for mt in range(ET):
                pkr = pp.tile([128, 1024], FP, tag="pG", name=f"pkr{mt}")
                for c0, cn in ((0, 512), (512, 488)):
                    for kt in range(ET):
                        nc.tensor.matmul(
                            pkr[:, c0:c0 + cn],
                            lhsT=wqk_sb[:, kt, ds(E + mt * 128, 128)],
                            rhs=embT_sb[:, kt, c0:c0 + cn],
                            start=(kt == 0), stop=(kt == ET - 1),
                            skip_group_check=True)
                nc.scalar.activation(
                    krelT_sb[:, mt, :], pkr[:, 0:R], AF.Identity,
                    bias=bqk_sb[:, ET + mt:ET + mt + 1])

        # initial x
        x_cur = xp.tile([128, ET, S, BL], FP, tag="x", name="x_init")
        for t in range(ET):
            nc.sync.dma_start(x_cur[:, t, :, :], d_x0[t])

        # ---------------- helpers ----------------
        h1s_dbg = [None]

        def gru_pairs(x_t, L, tag):
            """2-step GRU on all adjacent pairs of x_t ([128,ET,L,16]).
            Returns h2 tile [128, ET, P, BL] (pairs, j-major)."""
            P = L - 1
            cols = BL * L
            pcols = BL * P
            pG = pp.tile([128, 12, S, BL], FP, tag="pG", name=f"pG_{tag}")
            pN = pp.tile([128, ET, S, BL], FP, tag="pN", name=f"pN_{tag}")

            # G = x @ W_ih.T (+ bias row)  -> pG[:, m, 0:L, :]
            for m in range(12):
                for kt in range(ET):
                    nc.tensor.matmul(
                        pG[:, m, 0:L, :],
                        lhsT=wih_sb[:, kt, ds(m * 128, 128)],
                        rhs=x_t[:, kt, :, :],
                        start=(kt == 0 and m % 2 == 0), stop=False,
                        skip_group_check=True)
                nc.tensor.matmul(
                    pG[:, m, 0:L, :],
                    lhsT=browG_sb[:, ds(m * 128, 128)],
                    rhs=ones_r[:, 0:cols],
                    start=False, stop=(m >= 8),
                    skip_group_check=True)

            if debug_dump and L == S:
                for g3 in range(2):
                    dbg0 = gw.tile([128, ET, S - 1, BL], FP, tag="dbgn",
                                   name=f"dbg0_t{g3}")
                    nc.scalar.copy(dbg0[:],
                                   pG[:, 4 * g3:4 * (g3 + 1), 1:L, :])
                    nc.sync.dma_start(
                        d_dbg["dbg_pre0"][:, 4 * g3:4 * (g3 + 1)], dbg0[:])
            # ---- step 1 gates (h = 0); reads rows j=0..P-1 ----
            r1 = gw.tile([128, ET, P, BL], FP, tag="gA", name=f"r1_{tag}")
            z1c = gw.tile([128, ET, P, BL], FP, tag="gC", name=f"z1c_{tag}")
            t1 = gw.tile([128, ET, P, BL], FP, tag="gB", name=f"t1_{tag}")
            n1 = gw.tile([128, ET, P, BL], FP, tag="gD", name=f"n1_{tag}")
            h1s = gw.tile([128, ET, S, BL], FP, tag="gE", name=f"h1s_{tag}")

            nc.scalar.activation(r1[:], pG[:, 0:ET, 0:P, :], AF.Sigmoid)
            nc.scalar.activation(z1c[:], pG[:, ET:2 * ET, 0:P, :], AF.Sigmoid,
                                 scale=-1.0)
            for t in range(ET):
                nc.vector.scalar_tensor_tensor(
                    t1[:, t], r1[:, t], bnh_sb[:, t:t + 1],
                    pG[:, 8 + t, 0:P, :], op0=OP.mult, op1=OP.add)
            nc.scalar.activation(n1[:], t1[:], AF.Tanh)
            # h1 stored shifted: pair p at slot j=p+1
            nc.vector.tensor_tensor(h1s[:, :, 1:L, :], z1c[:], n1[:], OP.mult)
            h1s_dbg[0] = h1s

            # ---- gh2: h1 @ W_hh.T accumulated onto pG (r/z) and pN (n) ----
            for m in range(8):
                for kt in range(ET):
                    nc.tensor.matmul(
                        pG[:, m, 1:L, :],
                        lhsT=whh_sb[:, kt, ds(m * 128, 128)],
                        rhs=h1s[:, kt, 1:L, :],
                        start=False, stop=(kt == ET - 1),
                        skip_group_check=True)
            for t in range(ET):
                for kt in range(ET):
                    nc.tensor.matmul(
                        pN[:, t, 1:L, :],
                        lhsT=whh_sb[:, kt, ds((8 + t) * 128, 128)],
                        rhs=h1s[:, kt, 1:L, :],
                        start=(kt == 0 and t % 2 == 0), stop=False,
                        skip_group_check=True)
                nc.tensor.matmul(
                    pN[:, t, 1:L, :],
                    lhsT=brownh_sb[:, ds(t * 128, 128)],
                    rhs=ones_r[:, 0:pcols],
                    start=False, stop=True,
                    skip_group_check=True)

            # ---- step 2 gates; reads rows j=1..L-1 ----
            r2 = gw.tile([128, ET, P, BL], FP, tag="gA", name=f"r2_{tag}")
            z2 = gw.tile([128, ET, P, BL], FP, tag="gF", name=f"z2_{tag}")
            tt = gw.tile([128, ET, P, BL], FP, tag="gB", name=f"tt_{tag}")
            tt2 = gw.tile([128, ET, P, BL], FP, tag="gC", name=f"tt2_{tag}")
            n2 = gw.tile([128, ET, P, BL], FP, tag="gD", name=f"n2_{tag}")
            h2 = h2p.tile([128, ET, P, BL], FP, tag="h2", name=f"h2_{tag}")

            if debug_dump and L == S:
                for g3 in range(3):
                    dbgp = gw.tile([128, ET, S - 1, BL], FP, tag="dbgn",
                                   name=f"dbgp_t{g3}")
                    nc.scalar.copy(dbgp[:],
                                   pG[:, 4 * g3:4 * (g3 + 1), 1:L, :])
                    nc.sync.dma_start(
                        d_dbg["dbg_ppre"][:, 4 * g3:4 * (g3 + 1)], dbgp[:])
                dbgn = gw.tile([128, ET, S - 1, BL], FP, tag="dbgn",
                               name="dbgn_t")
                nc.scalar.copy(dbgn[:], pN[:, :, 1:L, :])
                nc.sync.dma_start(d_dbg["dbg_pn"][:], dbgn[:])
            nc.scalar.activation(r2[:], pG[:, 0:ET, 1:L, :], AF.Sigmoid)
            nc.scalar.activation(z2[:], pG[:, ET:2 * ET, 1:L, :], AF.Sigmoid)
            nc.vector.tensor_tensor(tt[:], r2[:], pN[:, :, 1:L, :], OP.mult)
            nc.vector.tensor_tensor(tt2[:], tt[:], pG[:, 8:12, 1:L, :],
                                    OP.add)
            nc.scalar.activation(n2[:], tt2[:], AF.Tanh)
            # h2 = n2 + z2*(h1 - n2);  h1 (unshifted) = h1s[:, :, 1:L, :]
            dd = gw.tile([128, ET, P, BL], FP, tag="gA", name=f"dd_{tag}")
            uu = gw.tile([128, ET, P, BL], FP, tag="gB", name=f"uu_{tag}")
            nc.vector.tensor_tensor(dd[:], h1s[:, :, 1:L, :], n2[:],
                                    OP.subtract)
            nc.vector.tensor_tensor(uu[:], z2[:], dd[:], OP.mult)
            nc.vector.tensor_tensor(h2[:], n2[:], uu[:], OP.add)
            return h2

        def attention(pair, tag, pA):
            """pair: [128, ET, BL] column-major selected vector.
            pA: [128, 3072] psum scratch (tag pG slot).
            Returns (ps, psxT, e_sb, ex, rZ, Zt, negm)."""
            ps = pA[:, 0:1024]
            pqk = pA[:, 1024:1152].rearrange("p (s b) -> p s b", b=BL)
            psx1 = pA[0:1, 1424:1488]
            prz = pA[0:1, 1488:1520]
            psxT = pA[0:BL, 1520:1521]
            for sl in range(2 * ET):
                for kt in range(ET):
                    nc.tensor.matmul(
                        pqk[:, sl, :],
                        lhsT=wqk_sb[:, kt, ds(sl * 128, 128)],
                        rhs=pair[:, kt, :],
                        start=(kt == 0), stop=(kt == ET - 1),
                        skip_group_check=True)
            qT = att.tile([128, ET, BL], FP, tag="qT", name=f"qT_{tag}")
            kxT = att.tile([128, ET, BL], FP, tag="kxT", name=f"kxT_{tag}")
            nc.vector.tensor_tensor(
                qT[:], pqk[:, 0:ET, :],
                bqk_sb[:, 0:ET].unsqueeze(-1).broadcast_to([128, ET, BL]),
                OP.add)
            nc.vector.tensor_tensor(
                kxT[:], pqk[:, ET:2 * ET, :],
                bqk_sb[:, ET:2 * ET].unsqueeze(-1).broadcast_to([128, ET, BL]),
                OP.add)

            # scores [BL, R] (q pre-scaled by 1/sqrt(E) on host)
            for c0, cn in ((0, 512), (512, 488)):
                for kt in range(ET):
                    nc.tensor.matmul(
                        ps[0:BL, c0:c0 + cn],
                        lhsT=qT[:, kt, :],
                        rhs=krelT_sb[:, kt, c0:c0 + cn],
                        start=(kt == 0), stop=(kt == ET - 1),
                        skip_group_check=True)
            # s_x[b] = q[b] . kx[b]
            sxm = att.tile([128, ET, BL], FP, tag="sxm", name=f"sxm_{tag}")
            nc.vector.tensor_tensor(sxm[:], qT[:], kxT[:], OP.mult)
            nc.tensor.matmul(
                psx1[:], lhsT=ones_c[:],
                rhs=sxm[:].rearrange("p a b -> p (a b)"),
                start=True, stop=True, skip_group_check=True)
            sx = att.tile([1, BL], FP, tag="sx", name=f"sx_{tag}")
            nc.vector.tensor_reduce(
                sx[:], psx1[:].rearrange("p (a b) -> p b a", a=ET),
                axis=AX.X, op=OP.add)
            nc.tensor.transpose(psxT, sx[:], ident[0:1, 0:1])

            # softmax pieces
            m1 = att.tile([BL, 1], FP, tag="m1", name=f"m1_{tag}")
            mm = att.tile([BL, 1], FP, tag="mm", name=f"mm_{tag}")
            negm = att.tile([BL, 1], FP, tag="negm", name=f"negm_{tag}")
            Z1 = att.tile([BL, 1], FP, tag="Z1", name=f"Z1_{tag}")
            Zt = att.tile([BL, 1], FP, tag="Zt", name=f"Zt_{tag}")
            ex = att.tile([BL, 1], FP, tag="ex", name=f"ex_{tag}")
            rZ = att.tile([BL, 1], FP, tag="rZ", name=f"rZ_{tag}")
            e_sb = att.tile([BL, R], FP, tag="e", bufs=1, name=f"e_{tag}")
            nc.vector.tensor_reduce(m1[:], ps[0:BL, 0:R], axis=AX.X, op=OP.max)
            nc.vector.tensor_tensor(mm[:], m1[:], psxT[:], OP.max)
            nc.vector.tensor_scalar_mul(negm[:], mm[:], -1.0)
            nc.scalar.activation(e_sb[:], ps[0:BL, 0:R], AF.Exp,
                                 bias=negm[:], accum_out=Z1[:])
            nc.scalar.activation(ex[:], psxT[:], AF.Exp, bias=negm[:])
            nc.vector.tensor_tensor(Zt[:], Z1[:], ex[:], OP.add)
            nc.vector.reciprocal(rZ[:], Zt[:])
            return ps, psxT, e_sb, ex, rZ, Zt, negm

        def loss_entries(ps, psxT, e_sb, ex, Zt, negm, it, tag):
            """Deferred entropy bookkeeping: Zs[:, it] = Z, Ts[:, it] = T
            where T = sum_j e_j * (s_j - m)."""
            d_sb = lsp.tile([BL, R], FP, tag="dls", name=f"dls_{tag}")
            scr = lsp.tile([BL, R], FP, tag="scr", name=f"scr_{tag}")
            T1 = lsp.tile([BL, 1], FP, tag="T1", name=f"T1_{tag}")
            dx = lsp.tile([BL, 1], FP, tag="dx", name=f"dx_{tag}")
            tx = lsp.tile([BL, 1], FP, tag="tx", name=f"tx_{tag}")
            Tt = lsp.tile([BL, 1], FP, tag="Tt", name=f"Tt_{tag}")
            nc.scalar.activation(d_sb[:], ps[0:BL, 0:R], AF.Identity,
                                 bias=negm[:])
            if debug_dump and tag == "L16":
                nc.sync.dma_start(d_dbg["dbg_d"][:], d_sb[:])
            nc.vector.scalar_tensor_tensor(
                scr[:], e_sb[:], 1.0, d_sb[:],
                op0=OP.mult, op1=OP.mult, accum_out=T1[:])
            nc.vector.tensor_tensor(dx[:], psxT[:], negm[:], OP.add)
            nc.vector.tensor_tensor(tx[:], ex[:], dx[:], OP.mult)
            nc.vector.tensor_tensor(Tt[:], T1[:], tx[:], OP.add)
            nc.scalar.copy(Zs[:, it:it + 1], Zt[:])
            nc.scalar.copy(Ts[:, it:it + 1], Tt[:])

        # ---------------- main loop ----------------
        it = 0
        for L in range(S, 2, -1):          # L = 16 .. 3 (full iterations)
            P = L - 1
            tag = f"L{L}"
            with nc.named_scope(f"iter_{tag}"):
                h2 = gru_pairs(x_cur, L, tag)

                # ---- selection: argmax over pair scores ----
                pA = pp.tile([128, 3072], FP, tag="pG", name=f"pA_{tag}")
                psc = pA[0:1, 1152:1408]
                for kt in range(ET):
                    nc.tensor.matmul(
                        psc[:, 0:BL * P],
                        lhsT=wfc_sb[:, kt:kt + 1],
                        rhs=h2[:, kt, :, :],
                        start=(kt == 0), stop=(kt == ET - 1),
                        skip_group_check=True)
                iotaS = selp.tile([128, P, BL], FP, tag="iotaS", bufs=2,
                                  name=f"io_{tag}")
                nc.gpsimd.iota(iotaS[:], pattern=[[1, P], [0, BL]],
                               base=-10000, channel_multiplier=0,
                               allow_small_or_imprecise_dtypes=True)
                scv = psc[:, 0:BL * P].rearrange("p (j b) -> p b j", b=BL)
                mB = selp.tile([1, BL], FP, tag="mB", name=f"mB_{tag}")
                oh = selp.tile([1, BL, P], FP, tag="oh", name=f"oh_{tag}")
                vv = selp.tile([1, BL, P], FP, tag="vv", name=f"vv_{tag}")
                sel = selp.tile([1, BL], FP, tag="sel", name=f"sel_{tag}")
                nc.vector.tensor_reduce(mB[:], scv, axis=AX.X, op=OP.max)
                nc.vector.tensor_tensor(
                    oh[:], scv, mB[:].unsqueeze(-1).broadcast_to([1, BL, P]),
                    OP.is_equal)
                nc.vector.tensor_tensor(
                    vv[:], oh[:], iotaS[0:1].rearrange("p j b -> p b j"),
                    OP.mult)
                nc.vector.tensor_reduce(sel[:], vv[:], axis=AX.X, op=OP.min)
                pselb = pA[:, 1408:1424]
                nc.tensor.matmul(pselb, lhsT=ones_r1[:], rhs=sel[:],
                                 start=True, stop=True,
                                 skip_group_check=True)
                mE = selp.tile([128, P, BL], FP, tag="mE", bufs=2,
                               name=f"mE_{tag}")
                mLT = selp.tile([128, P, BL], mybir.dt.uint8, tag="mLT",
                                bufs=2, name=f"mLT_{tag}")
                selbc = pselb.unsqueeze(1).broadcast_to([128, P, BL])
                nc.vector.tensor_tensor(mE[:], iotaS[:], selbc, OP.is_equal)
                nc.vector.tensor_tensor(mLT[:], iotaS[:], selbc, OP.is_lt)

                # pair gather: pair[:, t, b] = sum_j h2 * maskE
                pm = selp.tile([128, ET, P, BL], FP, tag="pm",
                               name=f"pm_{tag}")
                pair = selp.tile([128, ET, BL], FP, tag="pair", bufs=2,
                                 name=f"pr_{tag}")
                nc.vector.tensor_tensor(
                    pm[:], h2[:],
                    mE[:].unsqueeze(1).broadcast_to([128, ET, P, BL]),
                    OP.mult)
                nc.vector.tensor_reduce(
                    pair[:], pm[:].rearrange("p t j b -> p t b j"),
                    axis=AX.X, op=OP.add)

                # ---- compaction part 1 (prefetchable) ----
                x_new = xp.tile([128, ET, P, BL], FP, tag="x",
                                name=f"x_{tag}")
                nc.gpsimd.tensor_copy(x_new[:], x_cur[:, :, 1:L, :])
                for t in range(ET):
                    nc.vector.copy_predicated(
                        x_new[:, t], mLT[:], x_cur[:, t, 0:P, :])

                # ---- attention ----
                ps, psxT, e_sb, ex, rZ, Zt, negm = attention(pair, tag, pA)

                # merged.T = (emb_rel.T @ e.T + e_x * pair) / Z
                pB = pp.tile([128, 1024], FP, tag="pN", name=f"pB_{tag}")
                peT = pB[:, 0:128].rearrange("p (s b) -> p s b", b=BL)
                pmg = pB[:, 128:192].rearrange("p (s b) -> p s b", b=BL)
                pbc = pB[:, 192:224].rearrange("p (s b) -> p s b", b=BL)
                for sl in range(RT):
                    w = min(128, R - sl * 128)
                    nc.tensor.matmul(peT[0:w, sl, :],
                                     lhsT=e_sb[:, ds(sl * 128, w)],
                                     rhs=ident[0:BL, 0:BL],
                                     is_transpose=True, start=True,
                                     stop=True, skip_group_check=True)
                nc.vector.tensor_copy(eT_sb[:, 0:RT - 1, :],
                                      peT[:, 0:RT - 1, :])
                nc.vector.tensor_copy(eT_sb[0:104, RT - 1, :],
                                      peT[0:104, RT - 1, :])
                for mt in range(ET):
                    for kt8 in range(RT):
                        nc.tensor.matmul(
                            pmg[:, mt, :],
                            lhsT=embR_sb[:, kt8, ds(mt * 128, 128)],
                            rhs=eT_sb[:, kt8, :],
                            start=(kt8 == 0), stop=(kt8 == RT - 1),
                            skip_group_check=True)
                # broadcast 1/Z and e_x across partitions
                prz = pA[0:1, 1488:1520]
                nc.tensor.transpose(prz[:, 0:BL], rZ[:], ident[0:BL, 0:BL])
                nc.tensor.transpose(prz[:, BL:2 * BL], ex[:],
                                    ident[0:BL, 0:BL])
                rzT = att.tile([1, 2 * BL], FP, tag="rzT", name=f"rzT_{tag}")
                nc.vector.tensor_copy(rzT[:], prz[:])
                for sl in range(2):
                    nc.tensor.matmul(pbc[:, sl, :], lhsT=ones_r1[:],
                                     rhs=rzT[:, sl * BL:(sl + 1) * BL],
                                     start=True, stop=True,
                                     skip_group_check=True)
                mg1 = att.tile([128, ET, BL], FP, tag="mg1", name=f"m1_{tag}")
                mg2 = att.tile([128, ET, BL], FP, tag="mg2", name=f"m2_{tag}")
                merged = att.tile([128, ET, BL], FP, tag="mgd",
                                  name=f"mgd_{tag}")
                nc.vector.tensor_tensor(
                    mg1[:], pair[:],
                    pbc[:, 1:2, :].broadcast_to([128, ET, BL]), OP.mult)
                nc.vector.tensor_tensor(mg2[:], mg1[:], pmg[:], OP.add)
                nc.vector.tensor_tensor(
                    merged[:], mg2[:],
                    pbc[:, 0:1, :].broadcast_to([128, ET, BL]), OP.mult)

                # ---- compaction part 2: insert merged at sel ----
                # y = x_new + mE * (merged - x_new)   (ravel-safe TT ops)
                cd1 = gw.tile([128, ET, P, BL], FP, tag="gA", name=f"cd1_{tag}")
                cd2 = gw.tile([128, ET, P, BL], FP, tag="gB", name=f"cd2_{tag}")
                y_new = xp.tile([128, ET, P, BL], FP, tag="x",
                                name=f"y_{tag}")
                nc.vector.tensor_tensor(
                    cd1[:], merged[:].unsqueeze(2).broadcast_to(
                        [128, ET, P, BL]), x_new[:], OP.subtract)
                nc.vector.tensor_tensor(
                    cd2[:], cd1[:],
                    mE[:].unsqueeze(1).broadcast_to([128, ET, P, BL]),
                    OP.mult)
                nc.vector.tensor_tensor(y_new[:], x_new[:], cd2[:], OP.add)
                x_new = y_new

                # ---- deferred loss entries ----
                loss_entries(ps, psxT, e_sb, ex, Zt, negm, it, tag)

                if debug_dump and L == S:
                    nc.sync.dma_start(d_dbg["dbg_h1s"][:], h1s_dbg[0][:, :, 1:S, :])
                    nc.sync.dma_start(d_dbg["dbg_h2"][:], h2[:])
                    nc.sync.dma_start(d_dbg["dbg_sel"][:], sel[:])
                    nc.sync.dma_start(d_dbg["dbg_pair"][:], pair[:])
                    nc.sync.dma_start(d_dbg["dbg_e"][:], e_sb[:])
                    nc.sync.dma_start(d_dbg["dbg_mm"][:], negm[:])
                    nc.sync.dma_start(d_dbg["dbg_mgd"][:], merged[:])
                    nc.sync.dma_start(d_dbg["dbg_y"][:], x_new[:])
                x_cur = x_new
                it += 1

        # ------------- L=2 iteration (else branch; loss col 14 = 0) -------
        with nc.named_scope("iter_L2"):
            h2f = gru_pairs(x_cur, 2, "L2")       # [128, ET, 1, BL]

        # ---------------- final attention ----------------
        with nc.named_scope("final_att"):
            pairf = h2f[:].rearrange("p t j b -> p t (j b)")
            pAf = pp.tile([128, 3072], FP, tag="pG", name="pA_fin")
            ps, psxT, e_sb, ex, rZ, Zt, negm = attention(pairf, "fin", pAf)
            loss_entries(ps, psxT, e_sb, ex, Zt, negm, 15, "fin")
            scores_out = _tct([BL, R + 1], FP, name="scores_out")
            nc.scalar.copy(scores_out[:, 0:R], ps[0:BL, 0:R])
            nc.vector.tensor_copy(scores_out[:, R:R + 1], psxT[:])
            nc.sync.dma_start(d_scores[:], scores_out[:])

            # batched entropy: H = ln Z - T/Z
            lnZ = _tct([BL, S], FP, name="lnZ")
            rZs = _tct([BL, S], FP, name="rZs")
            ttm = _tct([BL, S], FP, name="ttm")
            nc.scalar.activation(lnZ[:], Zs[:], AF.Ln)
            nc.vector.reciprocal(rZs[:], Zs[:])
            nc.vector.tensor_tensor(ttm[:], Ts[:], rZs[:], OP.mult)
            nc.vector.tensor_tensor(losses_sb[:], lnZ[:], ttm[:], OP.subtract)
            nc.sync.dma_start(d_losses[:], losses_sb[:])

        _pools.close()

    nc.compile()
    return nc


def _host_prepare(inputs):
    """Build per-core in_maps (numpy, layout transforms only)."""
    tokens = np.asarray(inputs["tokens"])
    emb = np.asarray(inputs["emb"], dtype=np.float32)
    W_ih = np.asarray(inputs["W_ih"], dtype=np.float32)
    W_hh = np.asarray(inputs["W_hh"], dtype=np.float32)
    b_ih = np.asarray(inputs["b_ih"], dtype=np.float32)
    b_hh = np.asarray(inputs["b_hh"], dtype=np.float32)
    w_fc = np.asarray(inputs["w_fc"], dtype=np.float32)
    Wq = np.asarray(inputs["Wq"], dtype=np.float32)
    bq = np.asarray(inputs["bq"], dtype=np.float32)
    Wk = np.asarray(inputs["Wk"], dtype=np.float32)
    bk = np.asarray(inputs["bk"], dtype=np.float32)

    wihT = np.ascontiguousarray(W_ih.T).reshape(ET, 128, E3)
    whhT = np.ascontiguousarray(W_hh.T).reshape(ET, 128, E3)
    Wq_s = Wq * np.float32(SCALE)
    bq_s = bq * np.float32(SCALE)
    wqkT = np.ascontiguousarray(
        np.concatenate([Wq_s.T, Wk.T], axis=1)).reshape(ET, 128, 2 * E)
    embR = np.zeros((RPAD, E), np.float32)
    embR[:R] = emb[:R]
    embR = embR.reshape(RT, 128, E)
    embT = np.ascontiguousarray(emb[:R].T).reshape(ET, 128, R)

    browG = np.empty((1, E3), np.float32)
    browG[0, :2 * E] = (b_ih + b_hh)[:2 * E]
    browG[0, 2 * E:] = b_ih[2 * E:]
    brownh = b_hh[2 * E:].reshape(1, E).astype(np.float32).copy()
    bnhcol = np.ascontiguousarray(b_hh[2 * E:].reshape(ET, 128).T)
    bqk = np.ascontiguousarray(
        np.concatenate([bq_s.reshape(ET, 128), bk.reshape(ET, 128)],
                       axis=0).T)                          # [128, 2*ET]
    wfc = np.ascontiguousarray(w_fc.reshape(ET, 128).T)    # [128, ET]

    x0 = emb[tokens]                                       # [B, S, E]
    in_maps = []
    for c in range(NCORES):
        xc = x0[c * BL:(c + 1) * BL]                       # [BL, S, E]
        x0T = np.ascontiguousarray(
            xc.reshape(BL, S, ET, 128).transpose(2, 3, 1, 0))
        in_maps.append({
            "x0T": x0T, "wihT": wihT, "whhT": whhT, "wqkT": wqkT,
            "embR": embR, "embT": embT, "browG": browG, "brownh": brownh,
            "bnhcol": bnhcol, "bqk": bqk, "wfc": wfc,
        })
    return in_maps


def get_nc(debug_dump=False):
    key = ("nc", debug_dump)
    if key not in _CACHE:
        _CACHE[key] = _build(debug_dump)
    return _CACHE[key]


def kernel(**inputs):
    nc = get_nc(False)
    from concourse.bass_utils import run_bass_kernel_spmd
    in_maps = _host_prepare(inputs)
    res = run_bass_kernel_spmd(nc, in_maps, core_ids=list(range(NCORES)))
    scores = np.concatenate([r["scores"] for r in res.results], axis=0)
    losses = np.concatenate([r["losses"] for r in res.results], axis=0)
    return scores, losses
